# revision 17
# baseline (speedup 1.0000x reference)
"""Causal self-attention (B=4, S=2048, H=1024, 1 head) on 8 TRN2 NeuronCores.

Sharding: 8 cores = 4 batches x 2 query-groups. Core c handles batch b = c//2
and four 256-row query groups of that batch chosen so both cores of a batch do
equal attention work AND the uniform program's per-slot causal extents waste
as little as possible: g=0 owns groups {7,4,3,0} (extents 16,10,8,2), g=1 owns
{6,5,2,1} (extents 14,12,6,4); the program runs the elementwise max
(16,12,8,4) = 40 key-tile passes vs the 36 useful ones.

Precision plan (tolerance 2e-2; measured end-to-end ~1.5e-2 on the real inputs):
the TRN2 PE runs fp8 DoubleRow matmuls at 2x the bf16/f32r rate and fp8/bf16
weight loads pipeline behind the matmul stream (f32r loads do not), so
  - Q/K/V projections run in fp8e4m3 DoubleRow (weights pre-scaled by 8 so
    their ~N(0, 1/32) entries sit in e4m3's normal range; the 8 and the
    attention 1/sqrt(H) are divided back out in the PSUM->SBUF copies),
  - S^T, PV and the row-sum matmuls run in fp8 for slots 0-2 (qT/kT carry
    symmetric +-sqrt(32) scales so qT8.kT8 needs no correction),
  - EXCEPT rows with small causal support, where quantization noise cannot
    average out: slot 3 (the first 256 query rows of a g=0 core) and keys
    0-511 use bf16 end to end (Q/K/V projection, S^T, PV).  PSUM accumulation
    is fp32 everywhere; softmax (mask add + exp + final normalize) is fp32.
Only the last 4 key-tiles of each slot's extent can carry a nontrivial causal
mask (2 diagonal-crossing + up to 2 fully-masked waste tiles); the other 24
of 40 passes skip the mask add and exp straight out of PSUM.
Softmax skips max-subtraction (scores ~ N(0,1)), matching the reference.

Each core (uniform SPMD program, all per-core differences are input data):
  - projects Q for its 1024 query rows
  - streams keys in 2 phases of 1024: projects K^T and V for the phase,
    computes S^T = K^T-tiles x Q^T (scores transposed: k on partitions, q
    free), adds a host-provided causal mask, exp on ACT -> P^T, then PV +
    row-sum (ones-matmul) accumulate into SBUF.
  - normalizes by the row sums at the end and writes its [1024, 1024] output.
"""
import sys

sys.path.insert(0, "/opt/trn_rl_repo")

from contextlib import ExitStack

import numpy as np
import ml_dtypes

import concourse.bass as bass
import concourse.tile as tile
from concourse import bacc, bass_utils, mybir

F32 = mybir.dt.float32
BF16 = mybir.dt.bfloat16
FP8 = mybir.dt.float8e4
EXP = mybir.ActivationFunctionType.Exp
COPY = mybir.ActivationFunctionType.Copy
ADD = mybir.AluOpType.add
DR = mybir.MatmulPerfMode.DoubleRow
NP_BF16 = ml_dtypes.bfloat16
NP_FP8 = ml_dtypes.float8_e4m3

B, S, H = 4, 2048, 1024
N_CORES = 8
HO = H // 128          # 8 contraction subtiles
PH = 2                 # key phases
PHK = S // PH          # 1024 keys per phase
KT = PHK // 128        # 8 key tiles per phase
QL = 1024              # local query rows per core
NQT = QL // 128        # 8 query tiles of 128
NEG = -1.0e9
WS = 8.0               # fp8 weight pre-scale
QSC = 1.0 / (WS * 32.0)  # qT copy scale: undo WS, apply 1/sqrt(H)
KSC = 1.0 / WS
PH0_EXT = (8, 8, 8, 4)
PH1_EXT = (8, 4)
EXT_TOT = (16, 12, 8, 4)   # program extent per slot in global key-tiles
# only the last 4 key-tiles of each slot's extent can have a nontrivial causal
# mask (2 diagonal-crossing tiles + up to 2 fully-masked waste tiles); the
# rest are all-zero and skip the mask add entirely
N_MASK = 16
SC8 = 1.0 / (8.0 * 5.656854249)  # fp8 qT/kT copy scale (sqrt(32) balance)
# query groups (256 rows each, index i = rows [256i, 256(i+1)) of the batch)
# owned by pair-core g, in slot order (descending causal extent)
SLOT_GROUPS = {0: (7, 4, 3, 0), 1: (6, 5, 2, 1)}

_CACHE = {}


def _build(loop_t=None, unroll_t=1):
    nc = bacc.Bacc("TRN2", target_bir_lowering=False, debug=False,
                   num_devices=N_CORES)
    xqf_d = nc.dram_tensor("xq_f8", [128, HO, 768], FP8, kind="ExternalInput").ap()
    xqb_d = nc.dram_tensor("xq_bf", [128, HO, 256], BF16, kind="ExternalInput").ap()
    xkf_d = nc.dram_tensor("xkv_f8", [128, HO, S], FP8, kind="ExternalInput").ap()
    xkb_d = nc.dram_tensor("xkv_bf", [128, HO, 512], BF16, kind="ExternalInput").ap()
    wqb_d = nc.dram_tensor("wq_bf", [128, HO, H], BF16, kind="ExternalInput").ap()
    wkb_d = nc.dram_tensor("wk_bf", [128, HO, H], BF16, kind="ExternalInput").ap()
    wvb_d = nc.dram_tensor("wv_bf", [128, HO, H], BF16, kind="ExternalInput").ap()
    mask_d = nc.dram_tensor("masks", [N_MASK * 128, 256], BF16,
                            kind="ExternalInput").ap()
    onf_d = nc.dram_tensor("ones_f8", [128, 2], FP8, kind="ExternalInput").ap()
    onb_d = nc.dram_tensor("ones_bf", [128, 2], BF16, kind="ExternalInput").ap()
    o_d = nc.dram_tensor("o_out", [128, NQT, H], BF16, kind="ExternalOutput").ap()

    with tile.TileContext(nc) as tc, ExitStack() as ctx:
        if loop_t is not None:
            ctx.enter_context(tc.For_i(0, loop_t, 1))
        persist = ctx.enter_context(tc.tile_pool(name="persist", bufs=1))
        xspool = ctx.enter_context(tc.tile_pool(name="xspool", bufs=2))
        wpool = ctx.enter_context(tc.tile_pool(name="wpool", bufs=2))
        spool = ctx.enter_context(tc.tile_pool(name="spool", bufs=2))
        mpool = ctx.enter_context(tc.tile_pool(name="mpool", bufs=2))
        stpool = ctx.enter_context(tc.tile_pool(name="stpool", bufs=2))
        psum = ctx.enter_context(tc.tile_pool(name="psum", bufs=4, space="PSUM"))
        opsum = ctx.enter_context(tc.tile_pool(name="opsum", bufs=2, space="PSUM"))
        spsum = ctx.enter_context(tc.tile_pool(name="spsum", bufs=2, space="PSUM"))

        def mm_f8dr(ps, lhsT3, rhs3, nhp):
            """accumulate nhp DoubleRow matmuls (contraction pairs) into ps"""
            for hp in range(nhp):
                nc.tensor.matmul(ps, lhsT3(hp), rhs3(hp), perf_mode=DR,
                                 start=(hp == 0), stop=(hp == nhp - 1))

        for _t in range(unroll_t):
            qT8 = persist.tile([128, HO, 768], FP8, tag="qT8")
            qTb = persist.tile([128, HO, 256], BF16, tag="qTb")
            oacc = persist.tile([128, NQT, H], F32, tag="oacc")
            sums = persist.tile([128, NQT], F32, tag="sums")
            recip = persist.tile([128, NQT], F32, tag="recip")
            ones_f8 = persist.tile([128, 2], FP8, tag="ones_f8")
            ones_bf = persist.tile([128, 2], BF16, tag="ones_bf")
            wk8 = persist.tile([128, HO, H], FP8, tag="wk8")
            wv8 = persist.tile([128, HO, H], FP8, tag="wv8")

            nc.sync.dma_start(ones_f8[:], onf_d)
            nc.sync.dma_start(ones_bf[:], onb_d)
            _midx = [0]

            # ---- Q projection: qT[h, q] = (1/32) sum_h' wq[h', h] x^T[h', q].
            # fp8 DoubleRow for slots 0-2 (q cols 0:768), bf16 for slot 3.
            # first fp8 slice split across 4 DMA queues to shorten cold start
            xqf0 = xspool.tile([128, HO, 512], FP8, tag="xqf0")
            for h4 in range(4):
                nc.sync.dma_start(xqf0[:, 2 * h4:2 * h4 + 2, :],
                                  xqf_d[:, 2 * h4:2 * h4 + 2, 0:512])
            xqf1 = xspool.tile([128, HO, 256], FP8, tag="xqf1")
            nc.sync.dma_start(xqf1[:], xqf_d[:, :, 512:768])
            xqb = xspool.tile([128, HO, 256], BF16, tag="xqb")
            nc.sync.dma_start(xqb[:], xqb_d)
            for ht in range(HO):
                wtb = wpool.tile([128, HO, 128], BF16, tag="wtb")
                nc.sync.dma_start(wtb[:], wqb_d[:, :, ht * 128:(ht + 1) * 128])
                wtf = wpool.tile([128, HO, 128], FP8, tag="wtf")
                nc.gpsimd.tensor_copy(wtf[:], wtb[:])
                ps = psum.tile([128, 512], F32, tag="mm")
                mm_f8dr(ps[:], lambda hp: wtf[:, 2 * hp:2 * hp + 2, :],
                        lambda hp: xqf0[:, 2 * hp:2 * hp + 2, :], 4)
                nc.scalar.activation(qT8[:, ht, 0:512], ps[:], COPY, scale=SC8)
                ps = psum.tile([128, 256], F32, tag="mm")
                mm_f8dr(ps[:], lambda hp: wtf[:, 2 * hp:2 * hp + 2, :],
                        lambda hp: xqf1[:, 2 * hp:2 * hp + 2, :], 4)
                nc.scalar.activation(qT8[:, ht, 512:768], ps[:], COPY, scale=SC8)
                ps = psum.tile([128, 256], F32, tag="mm")
                for hs in range(HO):
                    nc.tensor.matmul(ps[:], wtb[:, hs, :], xqb[:, hs, :],
                                     start=(hs == 0), stop=(hs == HO - 1))
                nc.scalar.activation(qTb[:, ht, :], ps[:], COPY, scale=QSC)

            for ph in range(PH):
                # ---- K/V projection for this phase's keys ----
                # keys 0-511 (phase 0, kt 0-3) run bf16; the rest fp8 DR
                xhf = xspool.tile([128, HO, PHK], FP8, tag="xhf",
                                  name=f"xhf{ph}")
                for k4 in range(4):
                    nc.sync.dma_start(
                        xhf[:, 2 * k4:2 * k4 + 2, :],
                        xkf_d[:, 2 * k4:2 * k4 + 2, ph * PHK:(ph + 1) * PHK])
                if ph == 0:
                    xhb = xspool.tile([128, HO, 512], BF16, tag="xhb")
                    nc.sync.dma_start(xhb[:], xkb_d)
                kT8 = persist.tile([128, HO, PHK], FP8, tag="kT8", name=f"kT8_{ph}")
                if ph == 0:
                    kTb = persist.tile([128, HO, 512], BF16, tag="kTb")
                vT8 = persist.tile([128, KT, H], FP8, tag="vT8")
                if ph == 0:
                    vTb = persist.tile([128, 4, H], BF16, tag="vTb")
                for ht in range(HO):
                    if ph == 0:
                        wtb = wpool.tile([128, HO, 128], BF16, tag="wtb")
                        nc.sync.dma_start(wtb[:],
                                          wkb_d[:, :, ht * 128:(ht + 1) * 128])
                        nc.gpsimd.tensor_copy(wk8[:, :, ht * 128:(ht + 1) * 128],
                                              wtb[:])
                        ps = psum.tile([128, 512], F32, tag="mm")
                        for hs in range(HO):
                            nc.tensor.matmul(ps[:], wtb[:, hs, :], xhb[:, hs, :],
                                             start=(hs == 0), stop=(hs == HO - 1))
                        nc.vector.tensor_scalar_mul(kTb[:, ht, :], ps[:], KSC)
                        nc.vector.tensor_scalar_mul(kT8[:, ht, 0:512], ps[:], SC8)
                        ps = psum.tile([128, 512], F32, tag="mm")
                        mm_f8dr(ps[:],
                                lambda hp, ht=ht: wk8[:, 2 * hp:2 * hp + 2,
                                                      ht * 128:(ht + 1) * 128],
                                lambda hp: xhf[:, 2 * hp:2 * hp + 2, 512:1024], 4)
                        nc.vector.tensor_scalar_mul(kT8[:, ht, 512:1024], ps[:], SC8)
                    else:
                        for k2 in range(2):
                            ps = psum.tile([128, 512], F32, tag="mm")
                            mm_f8dr(ps[:],
                                    lambda hp, ht=ht: wk8[:, 2 * hp:2 * hp + 2,
                                                          ht * 128:(ht + 1) * 128],
                                    lambda hp, k2=k2: xhf[:, 2 * hp:2 * hp + 2,
                                                          k2 * 512:(k2 + 1) * 512], 4)
                            nc.vector.tensor_scalar_mul(
                                kT8[:, ht, k2 * 512:(k2 + 1) * 512], ps[:], SC8)
                for hh in range(2):
                    if ph == 0:
                        wvb = wpool.tile([128, HO, 512], BF16, tag="wvb")
                        nc.sync.dma_start(wvb[:],
                                          wvb_d[:, :, hh * 512:(hh + 1) * 512])
                        nc.gpsimd.tensor_copy(wv8[:, :, hh * 512:(hh + 1) * 512],
                                              wvb[:])
                    for kt in range(KT):
                        ps = psum.tile([128, 512], F32, tag="mm")
                        if ph == 0 and kt < 4:
                            for hs in range(HO):
                                nc.tensor.matmul(
                                    ps[:], xhb[:, hs, kt * 128:(kt + 1) * 128],
                                    wvb[:, hs, :],
                                    start=(hs == 0), stop=(hs == HO - 1))
                            nc.scalar.activation(
                                vTb[:, kt, hh * 512:(hh + 1) * 512], ps[:],
                                COPY, scale=KSC)
                        else:
                            mm_f8dr(ps[:],
                                    lambda hp, kt=kt: xhf[:, 2 * hp:2 * hp + 2,
                                                          kt * 128:(kt + 1) * 128],
                                    lambda hp, hh=hh: wv8[:, 2 * hp:2 * hp + 2,
                                                          hh * 512:(hh + 1) * 512], 4)
                        nc.scalar.activation(
                            vT8[:, kt, hh * 512:(hh + 1) * 512], ps[:],
                            COPY, scale=KSC)

                # ---- attention over this phase's keys ----
                exts = PH0_EXT if ph == 0 else PH1_EXT
                for qs, kts in enumerate(exts):
                    acc = qs == 3          # accurate (bf16) slot
                    pT = spool.tile([128, KT, 256], BF16 if acc else FP8,
                                    tag="pTb" if acc else "pT8")
                    for kt in range(kts):
                        gkt = ph * KT + kt
                        ps = psum.tile([128, 256], F32, tag="mm")
                        if acc:
                            for hs in range(HO):
                                nc.tensor.matmul(ps[:], kTb[:, hs, kt * 128:(kt + 1) * 128],
                                                 qTb[:, hs, :],
                                                 start=(hs == 0), stop=(hs == HO - 1))
                        else:
                            mm_f8dr(ps[:],
                                    lambda hp, kt=kt: kT8[:, 2 * hp:2 * hp + 2,
                                                          kt * 128:(kt + 1) * 128],
                                    lambda hp, qs=qs: qT8[:, 2 * hp:2 * hp + 2,
                                                          qs * 256:(qs + 1) * 256], 4)
                        if gkt >= EXT_TOT[qs] - 4:
                            mt = mpool.tile([128, 256], BF16, tag="mask")
                            nc.sync.dma_start(mt[:], mask_d[_midx[0] * 128:
                                                            (_midx[0] + 1) * 128, :])
                            _midx[0] += 1
                            sT = stpool.tile([128, 256], F32, tag="sT")
                            nc.vector.tensor_tensor(sT[:], ps[:], mt[:], ADD)
                            nc.scalar.activation(pT[:, kt, :], sT[:], EXP)
                        else:
                            nc.scalar.activation(pT[:, kt, :], ps[:], EXP)
                    for qi in range(2):
                        qt = qs * 2 + qi
                        for h2 in range(2):
                            po = opsum.tile([128, 512], F32, tag="o")
                            if acc:
                                for kt in range(kts):
                                    nc.tensor.matmul(
                                        po[:], pT[:, kt, qi * 128:(qi + 1) * 128],
                                        vTb[:, kt, h2 * 512:(h2 + 1) * 512],
                                        start=(kt == 0), stop=(kt == kts - 1))
                            else:
                                mm_f8dr(po[:],
                                        lambda kp, qi=qi: pT[:, 2 * kp:2 * kp + 2,
                                                             qi * 128:(qi + 1) * 128],
                                        lambda kp, h2=h2: vT8[:, 2 * kp:2 * kp + 2,
                                                              h2 * 512:(h2 + 1) * 512],
                                        kts // 2)
                            dst = oacc[:, qt, h2 * 512:(h2 + 1) * 512]
                            if ph == 0:
                                nc.vector.tensor_copy(dst, po[:])
                            else:
                                nc.vector.tensor_add(dst, dst, po[:])
                        pss = spsum.tile([128, 2], F32, tag="sum")
                        ones_t = ones_bf if acc else ones_f8
                        for kt in range(kts):
                            nc.tensor.matmul(pss[:], pT[:, kt, qi * 128:(qi + 1) * 128],
                                             ones_t[:],
                                             start=(kt == 0), stop=(kt == kts - 1))
                        dst = sums[:, qt:qt + 1]
                        if ph == 0:
                            nc.vector.tensor_copy(dst, pss[:, 0:1])
                        else:
                            nc.vector.tensor_add(dst, dst, pss[:, 0:1])

            # ---- normalize and write out (per q-tile, so the output DMA
            # overlaps the remaining normalization work) ----
            obf = persist.tile([128, NQT, H], BF16, tag="obf")
            for qt in range(NQT):
                nc.vector.reciprocal(recip[:, qt:qt + 1], sums[:, qt:qt + 1])
                nc.gpsimd.tensor_mul(obf[:, qt, :], oacc[:, qt, :],
                                     recip[:, qt:qt + 1].to_broadcast((128, H)))
                nc.sync.dma_start(o_d[:, qt, :], obf[:, qt, :])

    nc.compile()
    return nc


def _tile_hT(a):
    """[N, F] -> [128, N//128, F] with row n = (no*128 + p)."""
    n, f = a.shape
    return np.ascontiguousarray(a.reshape(n // 128, 128, f).transpose(1, 0, 2))


def _slot_starts(g):
    """Orig start rows of this core's four 256-row query groups, in slot
    order (descending causal extent)."""
    return [256 * i for i in SLOT_GROUPS[g]]


def _prep_core(x, w_qkv, b, g):
    xb = x[b]                                    # [S, H]
    starts = _slot_starts(g)
    xq = np.concatenate([xb[s:s + 256] for s in starts], axis=0)     # [QL, H]
    oq = np.concatenate([np.arange(s, s + 256) for s in starts])

    # each slot's causal extent must fit the uniform program's extent
    for slot, s in enumerate(starts):
        need = (s + 256) // 128
        have = PH0_EXT[slot] + (PH1_EXT[slot] if slot < len(PH1_EXT) else 0)
        assert need <= have, (g, slot, need, have)

    keys = np.arange(S)
    full = np.where(keys[:, None] <= oq[None, :], np.float32(0), np.float32(NEG))
    masks = np.empty((N_MASK, 128, 256), np.float32)
    i = 0
    for ph, exts in enumerate((PH0_EXT, PH1_EXT)):
        for qs, kts in enumerate(exts):
            for kt in range(kts):
                gkt = ph * KT + kt
                if gkt >= EXT_TOT[qs] - 4:
                    masks[i] = full[gkt * 128:(gkt + 1) * 128,
                                    qs * 256:(qs + 1) * 256]
                    i += 1
    assert i == N_MASK, i

    def f8(a):
        return np.ascontiguousarray(a).astype(NP_FP8)

    def bf(a):
        return np.ascontiguousarray(a).astype(NP_BF16)

    ws = np.float32(WS)
    xqT = xq.T                                   # [H, QL] slot order
    xbT = xb.T                                   # [H, S]
    return {
        "xq_f8": _tile_hT(f8(xqT[:, 0:768])),
        "xq_bf": _tile_hT(bf(xqT[:, 768:1024])),
        "xkv_f8": _tile_hT(f8(xbT)),
        "xkv_bf": _tile_hT(bf(xbT[:, 0:512])),
        "wq_bf": _tile_hT(bf(w_qkv[:, 0:H] * ws)),
        "wk_bf": _tile_hT(bf(w_qkv[:, H:2 * H] * ws)),
        "wv_bf": _tile_hT(bf(w_qkv[:, 2 * H:3 * H] * ws)),
        "masks": masks.reshape(N_MASK * 128, 256).astype(NP_BF16),
        "ones_f8": np.ones((128, 2), NP_FP8),
        "ones_bf": np.ones((128, 2), NP_BF16),
    }


def kernel(x, W_qkv, _trace=False, _trace_kwargs=None):
    x = np.asarray(x, np.float32)
    W_qkv = np.asarray(W_qkv, np.float32)
    if "nc" not in _CACHE:
        _CACHE["nc"] = _build()
    nc = _CACHE["nc"]

    in_maps = [_prep_core(x, W_qkv, c // 2, c % 2) for c in range(N_CORES)]
    kwargs = dict(_trace_kwargs or {})
    try:
        res = bass_utils.run_bass_kernel_spmd(
            nc, in_maps, core_ids=list(range(N_CORES)), trace=_trace, **kwargs)
    except Exception:
        # transient device wedge (e.g. NRT_EXEC_UNIT_UNRECOVERABLE) — retry once
        import time as _time
        _time.sleep(5)
        res = bass_utils.run_bass_kernel_spmd(
            nc, in_maps, core_ids=list(range(N_CORES)), trace=_trace, **kwargs)
    out = np.empty((B, S, H), np.float32)
    for c in range(N_CORES):
        b, g = c // 2, c % 2
        o = np.asarray(res.results[c]["o_out"], np.float32)
        o = o.transpose(1, 0, 2).reshape(QL, H)  # local q rows (slot order)
        for slot, s in enumerate(_slot_starts(g)):
            out[b, s:s + 256] = o[slot * 256:(slot + 1) * 256]
    _CACHE["last_results"] = res
    return out


if __name__ == "__main__":
    rng = np.random.default_rng(0)
    x = rng.standard_normal((B, S, H), dtype=np.float32)
    w = (rng.standard_normal((H, 3 * H)) / np.sqrt(H)).astype(np.float32)
    out = kernel(x, w)
    print("ran:", out.shape, out.dtype)


# revision 18
# speedup vs baseline: 1.1700x; 1.1700x over previous
"""Causal self-attention (B=4, S=2048, H=1024, 1 head) on 8 TRN2 NeuronCores.

Sharding: 8 cores = 4 batches x 2 query-groups. Core c handles batch b = c//2
and four 256-row query groups of that batch chosen so both cores of a batch do
equal attention work AND the uniform program's per-slot causal extents waste
as little as possible: g=0 owns groups {7,4,3,0} (extents 16,10,8,2), g=1 owns
{6,5,2,1} (extents 14,12,6,4); the program runs the elementwise max
(16,12,8,4) = 40 key-tile passes vs the 36 useful ones.

Precision plan (tolerance 2e-2; measured end-to-end ~1.5e-2 on the real inputs):
the TRN2 PE runs fp8 DoubleRow matmuls at 2x the bf16/f32r rate and fp8/bf16
weight loads pipeline behind the matmul stream (f32r loads do not), so
  - Q/K/V projections run in fp8e4m3 DoubleRow (weights pre-scaled by 8 so
    their ~N(0, 1/32) entries sit in e4m3's normal range; the 8 and the
    attention 1/sqrt(H) are divided back out in the PSUM->SBUF copies),
  - S^T, PV and the row-sum matmuls run in fp8 for slots 0-2 (qT/kT carry
    symmetric +-sqrt(32) scales so qT8.kT8 needs no correction),
  - EXCEPT rows with small causal support, where quantization noise cannot
    average out: slot 3 (the first 256 query rows of a g=0 core) and keys
    0-511 use bf16 end to end (Q/K/V projection, S^T, PV).  PSUM accumulation
    is fp32 everywhere; softmax (mask add + exp + final normalize) is fp32.
Only the last 4 key-tiles of each slot's extent can carry a nontrivial causal
mask (2 diagonal-crossing + up to 2 fully-masked waste tiles); the other 24
of 40 passes skip the mask add and exp straight out of PSUM.
Softmax skips max-subtraction (scores ~ N(0,1)), matching the reference.

Each core (uniform SPMD program, all per-core differences are input data):
  - projects Q for its 1024 query rows
  - streams keys in 2 phases of 1024: projects K^T and V for the phase,
    computes S^T = K^T-tiles x Q^T (scores transposed: k on partitions, q
    free), adds a host-provided causal mask, exp on ACT -> P^T, then PV +
    row-sum (ones-matmul) accumulate into SBUF.
  - normalizes by the row sums at the end and writes its [1024, 1024] output.
"""
import sys

sys.path.insert(0, "/opt/trn_rl_repo")

from contextlib import ExitStack

import numpy as np
import ml_dtypes

import concourse.bass as bass
import concourse.tile as tile
from concourse import bacc, bass_utils, mybir

F32 = mybir.dt.float32
BF16 = mybir.dt.bfloat16
FP8 = mybir.dt.float8e4
EXP = mybir.ActivationFunctionType.Exp
COPY = mybir.ActivationFunctionType.Copy
ADD = mybir.AluOpType.add
DR = mybir.MatmulPerfMode.DoubleRow
NP_BF16 = ml_dtypes.bfloat16
NP_FP8 = ml_dtypes.float8_e4m3

B, S, H = 4, 2048, 1024
N_CORES = 8
HO = H // 128          # 8 contraction subtiles
PH = 2                 # key phases
PHK = S // PH          # 1024 keys per phase
KT = PHK // 128        # 8 key tiles per phase
QL = 1024              # local query rows per core
NQT = QL // 128        # 8 query tiles of 128
NEG = -1.0e9
WS = 8.0               # fp8 weight pre-scale
QSC = 1.0 / (WS * 32.0)  # qT copy scale: undo WS, apply 1/sqrt(H)
KSC = 1.0 / WS
PH0_EXT = (8, 8, 8, 4)
PH1_EXT = (8, 4)
EXT_TOT = (16, 12, 8, 4)   # program extent per slot in global key-tiles
# only the last 4 key-tiles of each slot's extent can have a nontrivial causal
# mask (2 diagonal-crossing tiles + up to 2 fully-masked waste tiles); the
# rest are all-zero and skip the mask add entirely
N_MASK = 16
SC8 = 1.0 / (8.0 * 5.656854249)  # fp8 qT/kT copy scale (sqrt(32) balance)
# query groups (256 rows each, index i = rows [256i, 256(i+1)) of the batch)
# owned by pair-core g, in slot order (descending causal extent)
SLOT_GROUPS = {0: (7, 4, 3, 0), 1: (6, 5, 2, 1)}

_CACHE = {}


def _build(loop_t=None, unroll_t=1):
    nc = bacc.Bacc("TRN2", target_bir_lowering=False, debug=False,
                   num_devices=N_CORES)
    xqf_d = nc.dram_tensor("xq_f8", [128, HO, 768], FP8, kind="ExternalInput").ap()
    xqb_d = nc.dram_tensor("xq_bf", [128, HO, 256], BF16, kind="ExternalInput").ap()
    xkf_d = nc.dram_tensor("xkv_f8", [128, HO, S], FP8, kind="ExternalInput").ap()
    xkb_d = nc.dram_tensor("xkv_bf", [128, HO, 512], BF16, kind="ExternalInput").ap()
    wqf_d = nc.dram_tensor("wq_f8", [128, HO, H], FP8, kind="ExternalInput").ap()
    wkf_d = nc.dram_tensor("wk_f8", [128, HO, H], FP8, kind="ExternalInput").ap()
    wvf_d = nc.dram_tensor("wv_f8", [128, HO, H], FP8, kind="ExternalInput").ap()
    wqb_d = nc.dram_tensor("wq_bf", [128, HO, H], BF16, kind="ExternalInput").ap()
    wkb_d = nc.dram_tensor("wk_bf", [128, HO, H], BF16, kind="ExternalInput").ap()
    wvb_d = nc.dram_tensor("wv_bf", [128, HO, H], BF16, kind="ExternalInput").ap()
    mask_d = nc.dram_tensor("masks", [N_MASK * 128, 256], BF16,
                            kind="ExternalInput").ap()
    onf_d = nc.dram_tensor("ones_f8", [128, 2], FP8, kind="ExternalInput").ap()
    onb_d = nc.dram_tensor("ones_bf", [128, 2], BF16, kind="ExternalInput").ap()
    o_d = nc.dram_tensor("o_out", [128, NQT, H], BF16, kind="ExternalOutput").ap()

    with tile.TileContext(nc) as tc, ExitStack() as ctx:
        if loop_t is not None:
            ctx.enter_context(tc.For_i(0, loop_t, 1))
        persist = ctx.enter_context(tc.tile_pool(name="persist", bufs=1))
        xspool = ctx.enter_context(tc.tile_pool(name="xspool", bufs=2))
        wpool = ctx.enter_context(tc.tile_pool(name="wpool", bufs=2))
        spool = ctx.enter_context(tc.tile_pool(name="spool", bufs=2))
        mpool = ctx.enter_context(tc.tile_pool(name="mpool", bufs=2))
        stpool = ctx.enter_context(tc.tile_pool(name="stpool", bufs=2))
        psum = ctx.enter_context(tc.tile_pool(name="psum", bufs=4, space="PSUM"))
        opsum = ctx.enter_context(tc.tile_pool(name="opsum", bufs=2, space="PSUM"))
        spsum = ctx.enter_context(tc.tile_pool(name="spsum", bufs=2, space="PSUM"))

        def mm_f8dr(ps, lhsT3, rhs3, nhp):
            """accumulate nhp DoubleRow matmuls (contraction pairs) into ps"""
            for hp in range(nhp):
                nc.tensor.matmul(ps, lhsT3(hp), rhs3(hp), perf_mode=DR,
                                 start=(hp == 0), stop=(hp == nhp - 1))

        for _t in range(unroll_t):
            qT8 = persist.tile([128, HO, 768], FP8, tag="qT8")
            qTb = persist.tile([128, HO, 256], BF16, tag="qTb")
            oacc = persist.tile([128, NQT, H], F32, tag="oacc")
            sums = persist.tile([128, NQT], F32, tag="sums")
            recip = persist.tile([128, NQT], F32, tag="recip")
            ones_f8 = persist.tile([128, 2], FP8, tag="ones_f8")
            ones_bf = persist.tile([128, 2], BF16, tag="ones_bf")

            nc.sync.dma_start(ones_f8[:], onf_d)
            nc.sync.dma_start(ones_bf[:], onb_d)
            _midx = [0]

            # ---- Q projection: qT[h, q] = (1/32) sum_h' wq[h', h] x^T[h', q].
            # fp8 DoubleRow for slots 0-2 (q cols 0:768), bf16 for slot 3.
            # first fp8 slice split across 4 DMA queues to shorten cold start
            xqf0 = xspool.tile([128, HO, 512], FP8, tag="xqf0")
            for h4 in range(4):
                nc.sync.dma_start(xqf0[:, 2 * h4:2 * h4 + 2, :],
                                  xqf_d[:, 2 * h4:2 * h4 + 2, 0:512])
            xqf1 = xspool.tile([128, HO, 256], FP8, tag="xqf1")
            nc.sync.dma_start(xqf1[:], xqf_d[:, :, 512:768])
            xqb = xspool.tile([128, HO, 256], BF16, tag="xqb")
            nc.sync.dma_start(xqb[:], xqb_d)
            for ht in range(HO):
                wtf = wpool.tile([128, HO, 128], FP8, tag="wtf")
                nc.sync.dma_start(wtf[:], wqf_d[:, :, ht * 128:(ht + 1) * 128])
                wtb = wpool.tile([128, HO, 128], BF16, tag="wtb")
                nc.sync.dma_start(wtb[:], wqb_d[:, :, ht * 128:(ht + 1) * 128])
                ps = psum.tile([128, 512], F32, tag="mm")
                mm_f8dr(ps[:], lambda hp: wtf[:, 2 * hp:2 * hp + 2, :],
                        lambda hp: xqf0[:, 2 * hp:2 * hp + 2, :], 4)
                nc.scalar.activation(qT8[:, ht, 0:512], ps[:], COPY, scale=SC8)
                ps = psum.tile([128, 256], F32, tag="mm")
                mm_f8dr(ps[:], lambda hp: wtf[:, 2 * hp:2 * hp + 2, :],
                        lambda hp: xqf1[:, 2 * hp:2 * hp + 2, :], 4)
                nc.scalar.activation(qT8[:, ht, 512:768], ps[:], COPY, scale=SC8)
                ps = psum.tile([128, 256], F32, tag="mm")
                for hs in range(HO):
                    nc.tensor.matmul(ps[:], wtb[:, hs, :], xqb[:, hs, :],
                                     start=(hs == 0), stop=(hs == HO - 1))
                nc.scalar.activation(qTb[:, ht, :], ps[:], COPY, scale=QSC)

            for ph in range(PH):
                # ---- K/V projection for this phase's keys ----
                # keys 0-511 (phase 0, kt 0-3) run bf16; the rest fp8 DR
                xhf = xspool.tile([128, HO, PHK], FP8, tag="xhf",
                                  name=f"xhf{ph}")
                for k4 in range(4):
                    nc.sync.dma_start(
                        xhf[:, 2 * k4:2 * k4 + 2, :],
                        xkf_d[:, 2 * k4:2 * k4 + 2, ph * PHK:(ph + 1) * PHK])
                if ph == 0:
                    xhb = xspool.tile([128, HO, 512], BF16, tag="xhb")
                    nc.sync.dma_start(xhb[:], xkb_d)
                kT8 = persist.tile([128, HO, PHK], FP8, tag="kT8", name=f"kT8_{ph}")
                if ph == 0:
                    kTb = persist.tile([128, HO, 512], BF16, tag="kTb")
                vT8 = persist.tile([128, KT, H], FP8, tag="vT8")
                if ph == 0:
                    vTb = persist.tile([128, 4, H], BF16, tag="vTb")
                for ht in range(HO):
                    wtf = wpool.tile([128, HO, 128], FP8, tag="wtf")
                    nc.sync.dma_start(wtf[:], wkf_d[:, :, ht * 128:(ht + 1) * 128])
                    if ph == 0:
                        wtb = wpool.tile([128, HO, 128], BF16, tag="wtb")
                        nc.sync.dma_start(wtb[:],
                                          wkb_d[:, :, ht * 128:(ht + 1) * 128])
                        ps = psum.tile([128, 512], F32, tag="mm")
                        for hs in range(HO):
                            nc.tensor.matmul(ps[:], wtb[:, hs, :], xhb[:, hs, :],
                                             start=(hs == 0), stop=(hs == HO - 1))
                        nc.vector.tensor_scalar_mul(kTb[:, ht, :], ps[:], KSC)
                        nc.vector.tensor_scalar_mul(kT8[:, ht, 0:512], ps[:], SC8)
                        ps = psum.tile([128, 512], F32, tag="mm")
                        mm_f8dr(ps[:],
                                lambda hp: wtf[:, 2 * hp:2 * hp + 2, :],
                                lambda hp: xhf[:, 2 * hp:2 * hp + 2, 512:1024], 4)
                        nc.vector.tensor_scalar_mul(kT8[:, ht, 512:1024], ps[:], SC8)
                    else:
                        for k2 in range(2):
                            ps = psum.tile([128, 512], F32, tag="mm")
                            mm_f8dr(ps[:],
                                    lambda hp: wtf[:, 2 * hp:2 * hp + 2, :],
                                    lambda hp, k2=k2: xhf[:, 2 * hp:2 * hp + 2,
                                                          k2 * 512:(k2 + 1) * 512], 4)
                            nc.vector.tensor_scalar_mul(
                                kT8[:, ht, k2 * 512:(k2 + 1) * 512], ps[:], SC8)
                for hh in range(2):
                    wvf = wpool.tile([128, HO, 512], FP8, tag="wvf")
                    nc.sync.dma_start(wvf[:], wvf_d[:, :, hh * 512:(hh + 1) * 512])
                    if ph == 0:
                        wvb = wpool.tile([128, HO, 512], BF16, tag="wvb")
                        nc.sync.dma_start(wvb[:],
                                          wvb_d[:, :, hh * 512:(hh + 1) * 512])
                    for kt in range(KT):
                        ps = psum.tile([128, 512], F32, tag="mm")
                        if ph == 0 and kt < 4:
                            for hs in range(HO):
                                nc.tensor.matmul(
                                    ps[:], xhb[:, hs, kt * 128:(kt + 1) * 128],
                                    wvb[:, hs, :],
                                    start=(hs == 0), stop=(hs == HO - 1))
                            nc.scalar.activation(
                                vTb[:, kt, hh * 512:(hh + 1) * 512], ps[:],
                                COPY, scale=KSC)
                        else:
                            mm_f8dr(ps[:],
                                    lambda hp, kt=kt: xhf[:, 2 * hp:2 * hp + 2,
                                                          kt * 128:(kt + 1) * 128],
                                    lambda hp: wvf[:, 2 * hp:2 * hp + 2, :], 4)
                        nc.scalar.activation(
                            vT8[:, kt, hh * 512:(hh + 1) * 512], ps[:],
                            COPY, scale=KSC)

                # ---- attention over this phase's keys ----
                exts = PH0_EXT if ph == 0 else PH1_EXT
                for qs, kts in enumerate(exts):
                    acc = qs == 3          # accurate (bf16) slot
                    pT = spool.tile([128, KT, 256], BF16 if acc else FP8,
                                    tag="pTb" if acc else "pT8")
                    for kt in range(kts):
                        gkt = ph * KT + kt
                        ps = psum.tile([128, 256], F32, tag="mm")
                        if acc:
                            for hs in range(HO):
                                nc.tensor.matmul(ps[:], kTb[:, hs, kt * 128:(kt + 1) * 128],
                                                 qTb[:, hs, :],
                                                 start=(hs == 0), stop=(hs == HO - 1))
                        else:
                            mm_f8dr(ps[:],
                                    lambda hp, kt=kt: kT8[:, 2 * hp:2 * hp + 2,
                                                          kt * 128:(kt + 1) * 128],
                                    lambda hp, qs=qs: qT8[:, 2 * hp:2 * hp + 2,
                                                          qs * 256:(qs + 1) * 256], 4)
                        if gkt >= EXT_TOT[qs] - 4:
                            mt = mpool.tile([128, 256], BF16, tag="mask")
                            nc.sync.dma_start(mt[:], mask_d[_midx[0] * 128:
                                                            (_midx[0] + 1) * 128, :])
                            _midx[0] += 1
                            sT = stpool.tile([128, 256], F32, tag="sT")
                            nc.vector.tensor_tensor(sT[:], ps[:], mt[:], ADD)
                            nc.scalar.activation(pT[:, kt, :], sT[:], EXP)
                        else:
                            nc.scalar.activation(pT[:, kt, :], ps[:], EXP)
                    for qi in range(2):
                        qt = qs * 2 + qi
                        for h2 in range(2):
                            po = opsum.tile([128, 512], F32, tag="o")
                            if acc:
                                for kt in range(kts):
                                    nc.tensor.matmul(
                                        po[:], pT[:, kt, qi * 128:(qi + 1) * 128],
                                        vTb[:, kt, h2 * 512:(h2 + 1) * 512],
                                        start=(kt == 0), stop=(kt == kts - 1))
                            else:
                                mm_f8dr(po[:],
                                        lambda kp, qi=qi: pT[:, 2 * kp:2 * kp + 2,
                                                             qi * 128:(qi + 1) * 128],
                                        lambda kp, h2=h2: vT8[:, 2 * kp:2 * kp + 2,
                                                              h2 * 512:(h2 + 1) * 512],
                                        kts // 2)
                            dst = oacc[:, qt, h2 * 512:(h2 + 1) * 512]
                            if ph == 0:
                                nc.vector.tensor_copy(dst, po[:])
                            else:
                                nc.vector.tensor_add(dst, dst, po[:])
                        pss = spsum.tile([128, 2], F32, tag="sum")
                        ones_t = ones_bf if acc else ones_f8
                        for kt in range(kts):
                            nc.tensor.matmul(pss[:], pT[:, kt, qi * 128:(qi + 1) * 128],
                                             ones_t[:],
                                             start=(kt == 0), stop=(kt == kts - 1))
                        dst = sums[:, qt:qt + 1]
                        if ph == 0:
                            nc.vector.tensor_copy(dst, pss[:, 0:1])
                        else:
                            nc.vector.tensor_add(dst, dst, pss[:, 0:1])

            # ---- normalize and write out (per q-tile, so the output DMA
            # overlaps the remaining normalization work) ----
            obf = persist.tile([128, NQT, H], BF16, tag="obf")
            for qt in range(NQT):
                nc.vector.reciprocal(recip[:, qt:qt + 1], sums[:, qt:qt + 1])
                nc.gpsimd.tensor_mul(obf[:, qt, :], oacc[:, qt, :],
                                     recip[:, qt:qt + 1].to_broadcast((128, H)))
                nc.sync.dma_start(o_d[:, qt, :], obf[:, qt, :])

    nc.compile()
    return nc


def _tile_hT(a):
    """[N, F] -> [128, N//128, F] with row n = (no*128 + p)."""
    n, f = a.shape
    return np.ascontiguousarray(a.reshape(n // 128, 128, f).transpose(1, 0, 2))


def _slot_starts(g):
    """Orig start rows of this core's four 256-row query groups, in slot
    order (descending causal extent)."""
    return [256 * i for i in SLOT_GROUPS[g]]


def _prep_core(x, w_qkv, b, g):
    xb = x[b]                                    # [S, H]
    starts = _slot_starts(g)
    xq = np.concatenate([xb[s:s + 256] for s in starts], axis=0)     # [QL, H]
    oq = np.concatenate([np.arange(s, s + 256) for s in starts])

    # each slot's causal extent must fit the uniform program's extent
    for slot, s in enumerate(starts):
        need = (s + 256) // 128
        have = PH0_EXT[slot] + (PH1_EXT[slot] if slot < len(PH1_EXT) else 0)
        assert need <= have, (g, slot, need, have)

    keys = np.arange(S)
    full = np.where(keys[:, None] <= oq[None, :], np.float32(0), np.float32(NEG))
    masks = np.empty((N_MASK, 128, 256), np.float32)
    i = 0
    for ph, exts in enumerate((PH0_EXT, PH1_EXT)):
        for qs, kts in enumerate(exts):
            for kt in range(kts):
                gkt = ph * KT + kt
                if gkt >= EXT_TOT[qs] - 4:
                    masks[i] = full[gkt * 128:(gkt + 1) * 128,
                                    qs * 256:(qs + 1) * 256]
                    i += 1
    assert i == N_MASK, i

    def f8(a):
        return np.ascontiguousarray(a).astype(NP_FP8)

    def bf(a):
        return np.ascontiguousarray(a).astype(NP_BF16)

    ws = np.float32(WS)
    xqT = xq.T                                   # [H, QL] slot order
    xbT = xb.T                                   # [H, S]
    return {
        "xq_f8": _tile_hT(f8(xqT[:, 0:768])),
        "xq_bf": _tile_hT(bf(xqT[:, 768:1024])),
        "xkv_f8": _tile_hT(f8(xbT)),
        "xkv_bf": _tile_hT(bf(xbT[:, 0:512])),
        "wq_f8": _tile_hT(f8(w_qkv[:, 0:H] * ws)),
        "wk_f8": _tile_hT(f8(w_qkv[:, H:2 * H] * ws)),
        "wv_f8": _tile_hT(f8(w_qkv[:, 2 * H:3 * H] * ws)),
        "wq_bf": _tile_hT(bf(w_qkv[:, 0:H] * ws)),
        "wk_bf": _tile_hT(bf(w_qkv[:, H:2 * H] * ws)),
        "wv_bf": _tile_hT(bf(w_qkv[:, 2 * H:3 * H] * ws)),
        "masks": masks.reshape(N_MASK * 128, 256).astype(NP_BF16),
        "ones_f8": np.ones((128, 2), NP_FP8),
        "ones_bf": np.ones((128, 2), NP_BF16),
    }


def kernel(x, W_qkv, _trace=False, _trace_kwargs=None):
    x = np.asarray(x, np.float32)
    W_qkv = np.asarray(W_qkv, np.float32)
    if "nc" not in _CACHE:
        _CACHE["nc"] = _build()
    nc = _CACHE["nc"]

    in_maps = [_prep_core(x, W_qkv, c // 2, c % 2) for c in range(N_CORES)]
    kwargs = dict(_trace_kwargs or {})
    try:
        res = bass_utils.run_bass_kernel_spmd(
            nc, in_maps, core_ids=list(range(N_CORES)), trace=_trace, **kwargs)
    except Exception:
        # transient device wedge (e.g. NRT_EXEC_UNIT_UNRECOVERABLE) — retry once
        import time as _time
        _time.sleep(5)
        res = bass_utils.run_bass_kernel_spmd(
            nc, in_maps, core_ids=list(range(N_CORES)), trace=_trace, **kwargs)
    out = np.empty((B, S, H), np.float32)
    for c in range(N_CORES):
        b, g = c // 2, c % 2
        o = np.asarray(res.results[c]["o_out"], np.float32)
        o = o.transpose(1, 0, 2).reshape(QL, H)  # local q rows (slot order)
        for slot, s in enumerate(_slot_starts(g)):
            out[b, s:s + 256] = o[slot * 256:(slot + 1) * 256]
    _CACHE["last_results"] = res
    return out


if __name__ == "__main__":
    rng = np.random.default_rng(0)
    x = rng.standard_normal((B, S, H), dtype=np.float32)
    w = (rng.standard_normal((H, 3 * H)) / np.sqrt(H)).astype(np.float32)
    out = kernel(x, w)
    print("ran:", out.shape, out.dtype)


# revision 19
# speedup vs baseline: 1.3705x; 1.1714x over previous
"""Causal self-attention (B=4, S=2048, H=1024, 1 head) on 8 TRN2 NeuronCores.

Sharding: 8 cores = 4 batches x 2 query-groups. Core c handles batch b = c//2
and four 256-row query groups of that batch chosen so both cores of a batch do
equal attention work AND the uniform program's per-slot causal extents waste
as little as possible: g=0 owns groups {7,4,3,0} (extents 16,10,8,2), g=1 owns
{6,5,2,1} (extents 14,12,6,4); the program runs the elementwise max
(16,12,8,4) = 40 key-tile passes vs the 36 useful ones.

Precision plan (tolerance 2e-2; measured end-to-end ~1.5e-2 on the real inputs):
the TRN2 PE runs fp8 DoubleRow matmuls at 2x the bf16/f32r rate and fp8/bf16
weight loads pipeline behind the matmul stream (f32r loads do not), so
  - Q/K/V projections run in fp8e4m3 DoubleRow (weights pre-scaled by 8 so
    their ~N(0, 1/32) entries sit in e4m3's normal range; the 8 and the
    attention 1/sqrt(H) are divided back out in the PSUM->SBUF copies),
  - S^T, PV and the row-sum matmuls run in fp8 for slots 0-2 (qT/kT carry
    symmetric +-sqrt(32) scales so qT8.kT8 needs no correction),
  - EXCEPT rows with small causal support, where quantization noise cannot
    average out: slot 3 (the first 256 query rows of a g=0 core) and keys
    0-511 use bf16 end to end (Q/K/V projection, S^T, PV).  PSUM accumulation
    is fp32 everywhere; softmax (mask add + exp + final normalize) is fp32.
Only the last 4 key-tiles of each slot's extent can carry a nontrivial causal
mask (2 diagonal-crossing + up to 2 fully-masked waste tiles); the other 24
of 40 passes skip the mask add and exp straight out of PSUM.
Softmax skips max-subtraction (scores ~ N(0,1)), matching the reference.

Each core (uniform SPMD program, all per-core differences are input data):
  - projects Q for its 1024 query rows
  - streams keys in 2 phases of 1024: projects K^T and V for the phase,
    computes S^T = K^T-tiles x Q^T (scores transposed: k on partitions, q
    free), adds a host-provided causal mask, exp on ACT -> P^T, then PV +
    row-sum (ones-matmul) accumulate into SBUF.
  - normalizes by the row sums at the end and writes its [1024, 1024] output.
"""
import sys

sys.path.insert(0, "/opt/trn_rl_repo")

from contextlib import ExitStack

import numpy as np
import ml_dtypes

import concourse.bass as bass
import concourse.tile as tile
from concourse import bacc, bass_utils, mybir

F32 = mybir.dt.float32
BF16 = mybir.dt.bfloat16
FP8 = mybir.dt.float8e4
EXP = mybir.ActivationFunctionType.Exp
COPY = mybir.ActivationFunctionType.Copy
ADD = mybir.AluOpType.add
DR = mybir.MatmulPerfMode.DoubleRow
NP_BF16 = ml_dtypes.bfloat16
NP_FP8 = ml_dtypes.float8_e4m3

B, S, H = 4, 2048, 1024
N_CORES = 8
HO = H // 128          # 8 contraction subtiles
PH = 2                 # key phases
PHK = S // PH          # 1024 keys per phase
KT = PHK // 128        # 8 key tiles per phase
QL = 1024              # local query rows per core
NQT = QL // 128        # 8 query tiles of 128
NEG = -1.0e9
WS = 8.0               # fp8 weight pre-scale
QSC = 1.0 / (WS * 32.0)  # qT copy scale: undo WS, apply 1/sqrt(H)
KSC = 1.0 / WS
PH0_EXT = (8, 8, 8, 4)
PH1_EXT = (8, 4)
EXT_TOT = (16, 12, 8, 4)   # program extent per slot in global key-tiles
# only the last 4 key-tiles of each slot's extent can have a nontrivial causal
# mask (2 diagonal-crossing tiles + up to 2 fully-masked waste tiles); the
# rest are all-zero and skip the mask add entirely
N_MASK = 16
SC8 = 1.0 / (8.0 * 5.656854249)  # fp8 qT/kT copy scale (sqrt(32) balance)
# query groups (256 rows each, index i = rows [256i, 256(i+1)) of the batch)
# owned by pair-core g, in slot order (descending causal extent)
SLOT_GROUPS = {0: (7, 4, 3, 0), 1: (6, 5, 2, 1)}

_CACHE = {}


def _build(loop_t=None, unroll_t=1):
    nc = bacc.Bacc("TRN2", target_bir_lowering=False, debug=False,
                   num_devices=N_CORES)
    xqf_d = nc.dram_tensor("xq_f8", [128, HO, 768], FP8, kind="ExternalInput").ap()
    xqb_d = nc.dram_tensor("xq_bf", [128, HO, 256], BF16, kind="ExternalInput").ap()
    xkf_d = nc.dram_tensor("xkv_f8", [128, HO, S], FP8, kind="ExternalInput").ap()
    xkb_d = nc.dram_tensor("xkv_bf", [128, HO, 512], BF16, kind="ExternalInput").ap()
    wqf_d = nc.dram_tensor("wq_f8", [128, HO, H], FP8, kind="ExternalInput").ap()
    wkf_d = nc.dram_tensor("wk_f8", [128, HO, H], FP8, kind="ExternalInput").ap()
    wvf_d = nc.dram_tensor("wv_f8", [128, HO, H], FP8, kind="ExternalInput").ap()
    wqb_d = nc.dram_tensor("wq_bf", [128, HO, H], BF16, kind="ExternalInput").ap()
    wkb_d = nc.dram_tensor("wk_bf", [128, HO, H], BF16, kind="ExternalInput").ap()
    wvb_d = nc.dram_tensor("wv_bf", [128, HO, H], BF16, kind="ExternalInput").ap()
    mask_d = nc.dram_tensor("masks", [N_MASK * 128, 256], BF16,
                            kind="ExternalInput").ap()
    onf_d = nc.dram_tensor("ones_f8", [128, 2], FP8, kind="ExternalInput").ap()
    onb_d = nc.dram_tensor("ones_bf", [128, 2], BF16, kind="ExternalInput").ap()
    o_d = nc.dram_tensor("o_out", [128, NQT, H], BF16, kind="ExternalOutput").ap()

    with tile.TileContext(nc) as tc, ExitStack() as ctx:
        if loop_t is not None:
            ctx.enter_context(tc.For_i(0, loop_t, 1))
        persist = ctx.enter_context(tc.tile_pool(name="persist", bufs=1))
        xspool = ctx.enter_context(tc.tile_pool(name="xspool", bufs=2))
        wpool = ctx.enter_context(tc.tile_pool(name="wpool", bufs=3))
        spool = ctx.enter_context(tc.tile_pool(name="spool", bufs=2))
        mpool = ctx.enter_context(tc.tile_pool(name="mpool", bufs=2))
        stpool = ctx.enter_context(tc.tile_pool(name="stpool", bufs=2))
        psum = ctx.enter_context(tc.tile_pool(name="psum", bufs=4, space="PSUM"))
        opsum = ctx.enter_context(tc.tile_pool(name="opsum", bufs=2, space="PSUM"))
        spsum = ctx.enter_context(tc.tile_pool(name="spsum", bufs=2, space="PSUM"))

        def mm_f8dr(ps, lhsT3, rhs3, nhp):
            """accumulate nhp DoubleRow matmuls (contraction pairs) into ps"""
            for hp in range(nhp):
                nc.tensor.matmul(ps, lhsT3(hp), rhs3(hp), perf_mode=DR,
                                 start=(hp == 0), stop=(hp == nhp - 1))

        for _t in range(unroll_t):
            qT8 = persist.tile([128, HO, 768], FP8, tag="qT8")
            qTb = persist.tile([128, HO, 256], BF16, tag="qTb")
            oacc = persist.tile([128, NQT, H], F32, tag="oacc")
            sums = persist.tile([128, NQT], F32, tag="sums")
            recip = persist.tile([128, NQT], F32, tag="recip")
            ones_f8 = persist.tile([128, 2], FP8, tag="ones_f8")
            ones_bf = persist.tile([128, 2], BF16, tag="ones_bf")

            nc.sync.dma_start(ones_f8[:], onf_d)
            nc.sync.dma_start(ones_bf[:], onb_d)
            _midx = [0]

            # ---- Q projection: qT[h, q] = (1/32) sum_h' wq[h', h] x^T[h', q].
            # fp8 DoubleRow for slots 0-2 (q cols 0:768), bf16 for slot 3.
            # first fp8 slice split across 4 DMA queues to shorten cold start
            xqf0 = xspool.tile([128, HO, 512], FP8, tag="xqf0")
            for h4 in range(4):
                nc.sync.dma_start(xqf0[:, 2 * h4:2 * h4 + 2, :],
                                  xqf_d[:, 2 * h4:2 * h4 + 2, 0:512])
            xqf1 = xspool.tile([128, HO, 256], FP8, tag="xqf1")
            nc.sync.dma_start(xqf1[:], xqf_d[:, :, 512:768])
            xqb = xspool.tile([128, HO, 256], BF16, tag="xqb")
            nc.sync.dma_start(xqb[:], xqb_d)
            for ht in range(HO):
                wtf = wpool.tile([128, HO, 128], FP8, tag="wtf")
                nc.sync.dma_start(wtf[:], wqf_d[:, :, ht * 128:(ht + 1) * 128])
                wtb = wpool.tile([128, HO, 128], BF16, tag="wtb")
                nc.sync.dma_start(wtb[:], wqb_d[:, :, ht * 128:(ht + 1) * 128])
                ps = psum.tile([128, 512], F32, tag="mm")
                mm_f8dr(ps[:], lambda hp: wtf[:, 2 * hp:2 * hp + 2, :],
                        lambda hp: xqf0[:, 2 * hp:2 * hp + 2, :], 4)
                nc.scalar.activation(qT8[:, ht, 0:512], ps[:], COPY, scale=SC8)
                ps = psum.tile([128, 256], F32, tag="mm")
                mm_f8dr(ps[:], lambda hp: wtf[:, 2 * hp:2 * hp + 2, :],
                        lambda hp: xqf1[:, 2 * hp:2 * hp + 2, :], 4)
                nc.scalar.activation(qT8[:, ht, 512:768], ps[:], COPY, scale=SC8)
                ps = psum.tile([128, 256], F32, tag="mm")
                for hs in range(HO):
                    nc.tensor.matmul(ps[:], wtb[:, hs, :], xqb[:, hs, :],
                                     start=(hs == 0), stop=(hs == HO - 1))
                nc.scalar.activation(qTb[:, ht, :], ps[:], COPY, scale=QSC)

            for ph in range(PH):
                # ---- K/V projection for this phase's keys ----
                # keys 0-511 (phase 0, kt 0-3) run bf16; the rest fp8 DR
                xhf = xspool.tile([128, HO, PHK], FP8, tag="xhf",
                                  name=f"xhf{ph}")
                for k4 in range(4):
                    nc.sync.dma_start(
                        xhf[:, 2 * k4:2 * k4 + 2, :],
                        xkf_d[:, 2 * k4:2 * k4 + 2, ph * PHK:(ph + 1) * PHK])
                if ph == 0:
                    xhb = xspool.tile([128, HO, 512], BF16, tag="xhb")
                    nc.sync.dma_start(xhb[:], xkb_d)
                kT8 = persist.tile([128, HO, PHK], FP8, tag="kT8", name=f"kT8_{ph}")
                if ph == 0:
                    kTb = persist.tile([128, HO, 512], BF16, tag="kTb")
                vT8 = persist.tile([128, KT, H], FP8, tag="vT8")
                if ph == 0:
                    vTb = persist.tile([128, 4, H], BF16, tag="vTb")
                for ht in range(HO):
                    wtf = wpool.tile([128, HO, 128], FP8, tag="wtf")
                    nc.sync.dma_start(wtf[:], wkf_d[:, :, ht * 128:(ht + 1) * 128])
                    if ph == 0:
                        wtb = wpool.tile([128, HO, 128], BF16, tag="wtb")
                        nc.sync.dma_start(wtb[:],
                                          wkb_d[:, :, ht * 128:(ht + 1) * 128])
                        ps = psum.tile([128, 512], F32, tag="mm")
                        for hs in range(HO):
                            nc.tensor.matmul(ps[:], wtb[:, hs, :], xhb[:, hs, :],
                                             start=(hs == 0), stop=(hs == HO - 1))
                        nc.vector.tensor_scalar_mul(kTb[:, ht, :], ps[:], KSC)
                        nc.vector.tensor_scalar_mul(kT8[:, ht, 0:512], ps[:], SC8)
                        ps = psum.tile([128, 512], F32, tag="mm")
                        mm_f8dr(ps[:],
                                lambda hp: wtf[:, 2 * hp:2 * hp + 2, :],
                                lambda hp: xhf[:, 2 * hp:2 * hp + 2, 512:1024], 4)
                        nc.vector.tensor_scalar_mul(kT8[:, ht, 512:1024], ps[:], SC8)
                    else:
                        for k2 in range(2):
                            ps = psum.tile([128, 512], F32, tag="mm")
                            mm_f8dr(ps[:],
                                    lambda hp: wtf[:, 2 * hp:2 * hp + 2, :],
                                    lambda hp, k2=k2: xhf[:, 2 * hp:2 * hp + 2,
                                                          k2 * 512:(k2 + 1) * 512], 4)
                            nc.vector.tensor_scalar_mul(
                                kT8[:, ht, k2 * 512:(k2 + 1) * 512], ps[:], SC8)
                for hh in range(2):
                    wvf = wpool.tile([128, HO, 512], FP8, tag="wvf")
                    nc.sync.dma_start(wvf[:], wvf_d[:, :, hh * 512:(hh + 1) * 512])
                    if ph == 0:
                        wvb = wpool.tile([128, HO, 512], BF16, tag="wvb")
                        nc.sync.dma_start(wvb[:],
                                          wvb_d[:, :, hh * 512:(hh + 1) * 512])
                    for kt in range(KT):
                        ps = psum.tile([128, 512], F32, tag="mm")
                        if ph == 0 and kt < 4:
                            for hs in range(HO):
                                nc.tensor.matmul(
                                    ps[:], xhb[:, hs, kt * 128:(kt + 1) * 128],
                                    wvb[:, hs, :],
                                    start=(hs == 0), stop=(hs == HO - 1))
                            nc.scalar.activation(
                                vTb[:, kt, hh * 512:(hh + 1) * 512], ps[:],
                                COPY, scale=KSC)
                        else:
                            mm_f8dr(ps[:],
                                    lambda hp, kt=kt: xhf[:, 2 * hp:2 * hp + 2,
                                                          kt * 128:(kt + 1) * 128],
                                    lambda hp: wvf[:, 2 * hp:2 * hp + 2, :], 4)
                        nc.scalar.activation(
                            vT8[:, kt, hh * 512:(hh + 1) * 512], ps[:],
                            COPY, scale=KSC)

                # ---- attention over this phase's keys ----
                exts = PH0_EXT if ph == 0 else PH1_EXT
                for qs, kts in enumerate(exts):
                    acc = qs == 3          # accurate (bf16) slot
                    pT = spool.tile([128, KT, 256], BF16 if acc else FP8,
                                    tag="pTb" if acc else "pT8")
                    for kt in range(kts):
                        gkt = ph * KT + kt
                        ps = psum.tile([128, 256], F32, tag="mm")
                        if acc:
                            for hs in range(HO):
                                nc.tensor.matmul(ps[:], kTb[:, hs, kt * 128:(kt + 1) * 128],
                                                 qTb[:, hs, :],
                                                 start=(hs == 0), stop=(hs == HO - 1))
                        else:
                            mm_f8dr(ps[:],
                                    lambda hp, kt=kt: kT8[:, 2 * hp:2 * hp + 2,
                                                          kt * 128:(kt + 1) * 128],
                                    lambda hp, qs=qs: qT8[:, 2 * hp:2 * hp + 2,
                                                          qs * 256:(qs + 1) * 256], 4)
                        if gkt >= EXT_TOT[qs] - 4:
                            mt = mpool.tile([128, 256], BF16, tag="mask")
                            nc.sync.dma_start(mt[:], mask_d[_midx[0] * 128:
                                                            (_midx[0] + 1) * 128, :])
                            _midx[0] += 1
                            sT = stpool.tile([128, 256], F32, tag="sT")
                            nc.vector.tensor_tensor(sT[:], ps[:], mt[:], ADD)
                            nc.scalar.activation(pT[:, kt, :], sT[:], EXP)
                        else:
                            nc.scalar.activation(pT[:, kt, :], ps[:], EXP)
                    for qi in range(2):
                        qt = qs * 2 + qi
                        for h2 in range(2):
                            po = opsum.tile([128, 512], F32, tag="o")
                            if acc:
                                for kt in range(kts):
                                    nc.tensor.matmul(
                                        po[:], pT[:, kt, qi * 128:(qi + 1) * 128],
                                        vTb[:, kt, h2 * 512:(h2 + 1) * 512],
                                        start=(kt == 0), stop=(kt == kts - 1))
                            else:
                                mm_f8dr(po[:],
                                        lambda kp, qi=qi: pT[:, 2 * kp:2 * kp + 2,
                                                             qi * 128:(qi + 1) * 128],
                                        lambda kp, h2=h2: vT8[:, 2 * kp:2 * kp + 2,
                                                              h2 * 512:(h2 + 1) * 512],
                                        kts // 2)
                            dst = oacc[:, qt, h2 * 512:(h2 + 1) * 512]
                            if ph == 0:
                                nc.vector.tensor_copy(dst, po[:])
                            else:
                                nc.vector.tensor_add(dst, dst, po[:])
                        pss = spsum.tile([128, 2], F32, tag="sum")
                        ones_t = ones_bf if acc else ones_f8
                        for kt in range(kts):
                            nc.tensor.matmul(pss[:], pT[:, kt, qi * 128:(qi + 1) * 128],
                                             ones_t[:],
                                             start=(kt == 0), stop=(kt == kts - 1))
                        dst = sums[:, qt:qt + 1]
                        if ph == 0:
                            nc.vector.tensor_copy(dst, pss[:, 0:1])
                        else:
                            nc.vector.tensor_add(dst, dst, pss[:, 0:1])

            # ---- normalize and write out (per q-tile, so the output DMA
            # overlaps the remaining normalization work) ----
            obf = persist.tile([128, NQT, H], BF16, tag="obf")
            for qt in range(NQT):
                nc.vector.reciprocal(recip[:, qt:qt + 1], sums[:, qt:qt + 1])
                nc.gpsimd.tensor_mul(obf[:, qt, :], oacc[:, qt, :],
                                     recip[:, qt:qt + 1].to_broadcast((128, H)))
                nc.sync.dma_start(o_d[:, qt, :], obf[:, qt, :])

    nc.compile()
    return nc


def _tile_hT(a):
    """[N, F] -> [128, N//128, F] with row n = (no*128 + p)."""
    n, f = a.shape
    return np.ascontiguousarray(a.reshape(n // 128, 128, f).transpose(1, 0, 2))


def _slot_starts(g):
    """Orig start rows of this core's four 256-row query groups, in slot
    order (descending causal extent)."""
    return [256 * i for i in SLOT_GROUPS[g]]


def _prep_core(x, w_qkv, b, g):
    xb = x[b]                                    # [S, H]
    starts = _slot_starts(g)
    xq = np.concatenate([xb[s:s + 256] for s in starts], axis=0)     # [QL, H]
    oq = np.concatenate([np.arange(s, s + 256) for s in starts])

    # each slot's causal extent must fit the uniform program's extent
    for slot, s in enumerate(starts):
        need = (s + 256) // 128
        have = PH0_EXT[slot] + (PH1_EXT[slot] if slot < len(PH1_EXT) else 0)
        assert need <= have, (g, slot, need, have)

    keys = np.arange(S)
    full = np.where(keys[:, None] <= oq[None, :], np.float32(0), np.float32(NEG))
    masks = np.empty((N_MASK, 128, 256), np.float32)
    i = 0
    for ph, exts in enumerate((PH0_EXT, PH1_EXT)):
        for qs, kts in enumerate(exts):
            for kt in range(kts):
                gkt = ph * KT + kt
                if gkt >= EXT_TOT[qs] - 4:
                    masks[i] = full[gkt * 128:(gkt + 1) * 128,
                                    qs * 256:(qs + 1) * 256]
                    i += 1
    assert i == N_MASK, i

    def f8(a):
        return np.ascontiguousarray(a).astype(NP_FP8)

    def bf(a):
        return np.ascontiguousarray(a).astype(NP_BF16)

    ws = np.float32(WS)
    xqT = xq.T                                   # [H, QL] slot order
    xbT = xb.T                                   # [H, S]
    return {
        "xq_f8": _tile_hT(f8(xqT[:, 0:768])),
        "xq_bf": _tile_hT(bf(xqT[:, 768:1024])),
        "xkv_f8": _tile_hT(f8(xbT)),
        "xkv_bf": _tile_hT(bf(xbT[:, 0:512])),
        "wq_f8": _tile_hT(f8(w_qkv[:, 0:H] * ws)),
        "wk_f8": _tile_hT(f8(w_qkv[:, H:2 * H] * ws)),
        "wv_f8": _tile_hT(f8(w_qkv[:, 2 * H:3 * H] * ws)),
        "wq_bf": _tile_hT(bf(w_qkv[:, 0:H] * ws)),
        "wk_bf": _tile_hT(bf(w_qkv[:, H:2 * H] * ws)),
        "wv_bf": _tile_hT(bf(w_qkv[:, 2 * H:3 * H] * ws)),
        "masks": masks.reshape(N_MASK * 128, 256).astype(NP_BF16),
        "ones_f8": np.ones((128, 2), NP_FP8),
        "ones_bf": np.ones((128, 2), NP_BF16),
    }


def kernel(x, W_qkv, _trace=False, _trace_kwargs=None):
    x = np.asarray(x, np.float32)
    W_qkv = np.asarray(W_qkv, np.float32)
    if "nc" not in _CACHE:
        _CACHE["nc"] = _build()
    nc = _CACHE["nc"]

    in_maps = [_prep_core(x, W_qkv, c // 2, c % 2) for c in range(N_CORES)]
    kwargs = dict(_trace_kwargs or {})
    try:
        res = bass_utils.run_bass_kernel_spmd(
            nc, in_maps, core_ids=list(range(N_CORES)), trace=_trace, **kwargs)
    except Exception:
        # transient device wedge (e.g. NRT_EXEC_UNIT_UNRECOVERABLE) — retry once
        import time as _time
        _time.sleep(5)
        res = bass_utils.run_bass_kernel_spmd(
            nc, in_maps, core_ids=list(range(N_CORES)), trace=_trace, **kwargs)
    out = np.empty((B, S, H), np.float32)
    for c in range(N_CORES):
        b, g = c // 2, c % 2
        o = np.asarray(res.results[c]["o_out"], np.float32)
        o = o.transpose(1, 0, 2).reshape(QL, H)  # local q rows (slot order)
        for slot, s in enumerate(_slot_starts(g)):
            out[b, s:s + 256] = o[slot * 256:(slot + 1) * 256]
    _CACHE["last_results"] = res
    return out


if __name__ == "__main__":
    rng = np.random.default_rng(0)
    x = rng.standard_normal((B, S, H), dtype=np.float32)
    w = (rng.standard_normal((H, 3 * H)) / np.sqrt(H)).astype(np.float32)
    out = kernel(x, w)
    print("ran:", out.shape, out.dtype)
